# revision 1
# baseline (speedup 1.0000x reference)
"""Trainium2 Bass kernel for nn_DFlashSelfAttention (block-sparse GQA attention).

Self-contained: builds the Bass module once, shards inputs over 8 NeuronCores
(sequence-parallel), runs via run_bass_kernel_spmd, reassembles full output.
"""

import sys as _sys
for _p in ("/opt/trn_rl_repo",):
    if _p not in _sys.path:
        _sys.path.insert(0, _p)

"""Bass/Tile kernel for DFlashSelfAttention (block-diagonal causal attention).

Sharding: sequence-parallel over L (2048 -> 8 cores x 256 positions).
Attention is block-diagonal with BLOCK=16, so positions never interact
across 16-blocks; a 256-position slice (16 blocks) is fully independent.

Per-core pipeline (T = 512 rows = 2 batches x 256 positions):
  phase B: KV = X @ Wkv        (k_nat f32, v fp16)
  k-rope:  RMS-norm + RoPE on K for all chunks (f32 math, fp16 out)
  phase A: Q  = X @ Wq
  per 128-chunk: q-rope, PE-transpose to [d, t], then:
    stage 1 (16 heads): mask preloaded into S psum via fp16 matmul
      (identity x mask), S = qT.T @ kT accumulated on top, exp on ACT from
      PSUM with accumulated row sums.
    stage 2 (head pairs): normalize on GPSIMD (fp16 out), PE-transpose P,
      copy back, outT = v.T @ P^T, A^T copies (odd heads relayouted by DMA).
  out:     Y^T = Wo^T @ A^T -> fp16 DRAM [4096, 512]; host transposes back.

All matmul operands are fp16 (1 cy/row, ~5e-4 quantization); accumulation
and all softmax/norm arithmetic stay fp32. RMS-norm weights and the
sqrt(1/8) attention scale are folded into host-precomputed rope tables.
"""

import ml_dtypes
import numpy as np

import concourse.bass as bass
import concourse.mybir as mybir
import concourse.tile as tile
from concourse import bacc
from concourse.masks import make_identity

F32 = mybir.dt.float32
F16 = mybir.dt.float16

P = 128
HID = 4096
KO = HID // P          # 32 k-chunks over hidden
T = 512                # rows per core: 2 batches x 256 positions
NM = T // P            # 4 t-chunks
NH = 16
NKV = 4
HD = 64
QD = NH * HD           # 1024
KVD = 2 * NKV * HD     # 512 (k 256 | v 256)
EPS = 1e-6

# Q-head permutation: position p holds original head PERM[p]. Even positions
# carry heads whose KV head is even (partition half 0), odd positions heads
# with odd KV head (half 64) — so S-matmul operands share a base partition
# and head pairs run on disjoint PE row groups concurrently.
PERM = [0, 4, 1, 5, 2, 6, 3, 7, 8, 12, 9, 13, 10, 14, 11, 15]


def build_nc(name="dfa"):
    nc = bacc.Bacc(None, target_bir_lowering=False, name=name)

    xt = nc.dram_tensor("xt", [HID, T], F16, kind="ExternalInput")
    wq = nc.dram_tensor("wq", [HID, QD], F16, kind="ExternalInput")
    wkv = nc.dram_tensor("wkv", [HID, KVD], F16, kind="ExternalInput")
    wo = nc.dram_tensor("wo", [32, P, 8, P], F16, kind="ExternalInput")
    cwq = nc.dram_tensor("cwq", [T, HD], F32, kind="ExternalInput")
    swq = nc.dram_tensor("swq", [T, HD], F32, kind="ExternalInput")
    cwk = nc.dram_tensor("cwk", [T, HD], F32, kind="ExternalInput")
    swk = nc.dram_tensor("swk", [T, HD], F32, kind="ExternalInput")
    mask = nc.dram_tensor("mask", [P, P], F16, kind="ExternalInput")
    yt = nc.dram_tensor("yt", [HID, T], F16, kind="ExternalOutput")

    from contextlib import ExitStack
    with tile.TileContext(nc) as tc, ExitStack() as ctx:
        consts = ctx.enter_context(tc.tile_pool(name="consts", bufs=1))
        xt_pool = ctx.enter_context(tc.tile_pool(name="xt", bufs=1))
        wstream = ctx.enter_context(tc.tile_pool(name="wstream", bufs=4))
        acts = ctx.enter_context(tc.tile_pool(name="acts", bufs=1))
        rope_tmp = ctx.enter_context(tc.tile_pool(name="rope_tmp", bufs=1))
        tr_pool = ctx.enter_context(tc.tile_pool(name="tr", bufs=1))
        attn_tmp = ctx.enter_context(tc.tile_pool(name="attn_tmp", bufs=2))
        ystage = ctx.enter_context(tc.tile_pool(name="ystage", bufs=2))
        pp = ctx.enter_context(tc.tile_pool(name="pp", bufs=1, space="PSUM"))

        def ptile(shape, bank, name, dtype=F32):
            return pp.tile(shape, dtype, tag=f"b{bank}", name=name,
                           padded_shape=[P, 512])

        # ---- leading DMAs: first wkv batch + x chunks feed phase B ----
        xt_sb = xt_pool.tile([P, KO, T], F16)
        xt_r = xt.rearrange("(ko p) t -> p ko t", p=P)
        wkv_r = wkv.rearrange("(kb p) n -> p kb n", p=P)
        wq_r = wq.rearrange("(kb p) n -> p kb n", p=P)
        wkv_tiles = {0: wstream.tile([P, 4, KVD], F16, tag="wkv",
                                     name="wkv_k0")}
        nc.sync.dma_start(wkv_tiles[0][:], wkv_r[:, 0:4, :])
        nc.sync.dma_start(xt_sb[:, 0:4, :], xt_r[:, 0:4, :])
        for q in range(1, 8):
            nc.sync.dma_start(xt_sb[:, q * 4:(q + 1) * 4, :],
                              xt_r[:, q * 4:(q + 1) * 4, :])

        # ---- constants (small DMAs ride behind the big leading ones) ----
        ident = consts.tile([P, P], F16)
        make_identity(nc, ident)
        eps_t = consts.tile([P, 1], F32)
        nc.vector.memset(eps_t, EPS)
        mask_sb = consts.tile([P, P], F16)
        nc.sync.dma_start(mask_sb[:], mask[:])
        tabs = {}
        for nm_, dr_ in (("cwq", cwq), ("swq", swq), ("cwk", cwk),
                         ("swk", swk)):
            tt = consts.tile([P, NM, HD], F32, tag=nm_, name=nm_)
            nc.sync.dma_start(tt[:], dr_.rearrange("(m p) d -> p m d", p=P))
            tabs[nm_] = tt

        # ---- phase B: KV = X @ Wkv ----
        ps_b = [ptile([P, KVD], m, f"psb{m}") for m in range(NM)]
        for kb in range(8):
            if kb not in wkv_tiles:
                wkv_tiles[kb] = wstream.tile([P, 4, KVD], F16, tag="wkv",
                                             name=f"wkv_k{kb}")
                nc.sync.dma_start(wkv_tiles[kb][:],
                                  wkv_r[:, kb * 4:(kb + 1) * 4, :])
            wkv_k = wkv_tiles[kb]
            for kk in range(4):
                k = kb * 4 + kk
                for m in range(NM):
                    nc.tensor.matmul(
                        ps_b[m][:], xt_sb[:, k, m * P:(m + 1) * P],
                        wkv_k[:, kk, :],
                        start=(k == 0), stop=(k == KO - 1))

        k_nat = []
        v_sb = []
        for m in range(NM):
            kn = acts.tile([P, NKV, HD], F32, tag="knat", name=f"knat{m}",
                           bufs=2)
            nc.vector.tensor_copy(
                kn[:], ps_b[m][:, 0:256].rearrange("p (h d) -> p h d", d=HD))
            k_nat.append(kn)
            vv = acts.tile([P, 256], F16, tag=f"v{m}", name=f"v{m}")
            nc.vector.tensor_copy(vv[:], ps_b[m][:, 256:512])
            v_sb.append(vv)

        def rope_stats(src_t, nh, tag):
            """x^2 sum -> rstd [P, nh] f32 (ACT Sqrt era)."""
            sq = rope_tmp.tile([P, nh, HD], F32, tag=f"sq{nh}", name="sq",
                               bufs=2)
            nc.vector.tensor_mul(sq[:], src_t[:], src_t[:])
            rstd = rope_tmp.tile([P, nh], F32, tag=tag, bufs=4, name=tag)
            nc.vector.reduce_sum(rstd[:], sq[:], axis=mybir.AxisListType.X)
            nc.scalar.activation(rstd[:], rstd[:],
                                 mybir.ActivationFunctionType.Sqrt,
                                 bias=eps_t[:], scale=1.0 / HD)
            nc.vector.reciprocal(rstd[:], rstd[:])
            return rstd

        def rope_apply(src_t, rstd, m, nh, ctab, stab, out_tag):
            """(src*rstd) rotary -> fp16 tile [P, nh*HD]. DVE/Pool only."""
            qn = rope_tmp.tile([P, nh, HD], F32, tag=f"qn{nh}", name="qn")
            nc.vector.tensor_mul(qn[:], src_t[:],
                                 rstd[:, :, None].to_broadcast((P, nh, HD)))
            o1 = rope_tmp.tile([P, nh, HD], F32, tag=f"o1{nh}", name="o1")
            nc.vector.tensor_mul(o1[:], qn[:],
                                 ctab[:, m, None, :].to_broadcast((P, nh, HD)))
            o2 = rope_tmp.tile([P, nh, HD], F32, tag=f"o2{nh}", name="o2")
            H2 = HD // 2
            nc.gpsimd.tensor_mul(
                o2[:, :, 0:H2], qn[:, :, H2:HD],
                stab[:, m, None, 0:H2].to_broadcast((P, nh, H2)))
            nc.gpsimd.tensor_mul(
                o2[:, :, H2:HD], qn[:, :, 0:H2],
                stab[:, m, None, H2:HD].to_broadcast((P, nh, H2)))
            outt = rope_tmp.tile([P, nh * HD], F16, tag=out_tag, bufs=2,
                                 name=out_tag)
            nc.vector.tensor_add(
                outt[:], o1[:].rearrange("p h d -> p (h d)"),
                o2[:].rearrange("p h d -> p (h d)"))
            return outt

        # k-rope + krT transposes for ALL chunks now (overlaps phase A)
        krT = [[tr_pool.tile([P, P], F16, tag=f"krT{m}_{g2}", bufs=1,
                             name=f"krT{m}_{g2}")
                for g2 in range(2)] for m in range(NM)]
        for m in range(NM):
            k_rstd = rope_stats(k_nat[m], NKV, f"krstd{m}")
            kr = rope_apply(k_nat[m], k_rstd, m, NKV, tabs["cwk"],
                            tabs["swk"], "krout")
            for g2 in range(2):
                pt = ptile([P, P], g2, f"trk{m}_{g2}", F16)
                nc.tensor.matmul(pt[:], kr[:, g2 * P:(g2 + 1) * P],
                                 ident[:], is_transpose=True)
                nc.vector.tensor_copy(krT[m][g2][:], pt[:])

        # ---- phase A: Q = X @ Wq, split into two half-passes so the
        # attention chains of chunks 0-1 overlap the matmuls of half 2. ----
        ps_a = [[ptile([P, 512], (2 * m + s) % 8, f"psa{m}_{s}")
                 for s in range(2)] for m in range(NM)]

        def phase_a_half(ms):
            for kb in range(8):
                wq_k = wstream.tile([P, 4, QD], F16, tag="wq",
                                    name=f"wq_k{ms[0]}_{kb}")
                nc.sync.dma_start(wq_k[:], wq_r[:, kb * 4:(kb + 1) * 4, :])
                for kk in range(4):
                    k = kb * 4 + kk
                    for m in ms:
                        for s in range(2):
                            nc.tensor.matmul(
                                ps_a[m][s][:],
                                xt_sb[:, k, m * P:(m + 1) * P],
                                wq_k[:, kk, s * 512:(s + 1) * 512],
                                start=(k == 0), stop=(k == KO - 1))

        # A^T accumulator [P, 8, T]: col block kk holds permuted positions
        # (2kk, 2kk+1); partition = (pos%2)*64 + d; col t.
        at_sb = acts.tile([P, 8, T], F16, tag="at")

        # ones vectors for softmax column sums / broadcast
        ones_c = consts.tile([P, 1], F16)
        nc.vector.memset(ones_c, 1.0)
        ones_r = consts.tile([1, HD], F32)
        nc.vector.memset(ones_r, 1.0)

        def q_stats(m):
            qn_t = acts.tile([P, NH, HD], F32, tag="qnat", name=f"qnat{m}",
                             bufs=4)
            for s in range(2):
                nc.vector.tensor_copy(
                    qn_t[:, s * 8:(s + 1) * 8, :],
                    ps_a[m][s][:].rearrange("p (h d) -> p h d", d=HD))
            return qn_t, rope_stats(qn_t, NH, f"qrstd{m}")

        def chunk_pre(m, qn_t, rstd, trb):
            """rope-apply + qrT transposes for chunk m."""
            qr = rope_apply(qn_t, rstd, m, NH, tabs["cwq"], tabs["swq"],
                            "qrout")
            qrT = [tr_pool.tile([P, P], F16, tag=f"qrT{hh}", bufs=2,
                                name=f"qrT{m}_{hh}") for hh in range(8)]
            for hh in range(8):
                pt = ptile([P, P], trb[hh % len(trb)], f"trq{m}_{hh}", F16)
                nc.tensor.matmul(pt[:], qr[:, hh * P:(hh + 1) * P],
                                 ident[:], is_transpose=True)
                nc.vector.tensor_copy(qrT[hh][:], pt[:])
            odd_stage = attn_tmp.tile([HD, 8, P], F16, bufs=1,
                                      tag="odd_stage", name=f"odd_stage{m}")
            return qrT, odd_stage

        def attn_pos(m, qrT, odd_stage, pos, stb, csb):
            h = PERM[pos]
            g = h // 4
            base = (pos % 2) * HD
            hh = pos // 2
            lq = qrT[hh][base:base + HD, :]
            lk = krT[m][g // 2][base:base + HD, :]
            sb = stb[pos % len(stb)]
            cb = csb[pos % len(csb)]
            st_ps = ptile([P, P], sb, f"st_ps{sb}")
            nc.tensor.matmul(st_ps[:], ident[:], mask_sb[:],
                             start=True, stop=False)
            nc.tensor.matmul(st_ps[:], lk, lq, start=False, stop=True)
            est = attn_tmp.tile([P, P], F16, tag=f"est{pos % 8}", bufs=3,
                                name=f"est{pos % 8}")
            nc.scalar.activation(est[:], st_ps[:],
                                 mybir.ActivationFunctionType.Exp)
            csum_ps = ptile([1, P], cb, f"csum{cb}")
            nc.tensor.matmul(csum_ps[:], ones_c[:], est[:])
            rs = attn_tmp.tile([1, P], F32, tag=f"rs{pos % 4}", bufs=2,
                               name=f"rs{pos % 4}")
            nc.vector.reciprocal(rs[:], csum_ps[:])
            rsb_ps = ptile([HD, P], cb, f"rsb{cb}")
            nc.tensor.matmul(rsb_ps[:], ones_r[:], rs[:])
            rsb_sb = attn_tmp.tile([HD, P], F32, tag=f"rsb_sb{pos % 4}",
                                   bufs=2, name=f"rsb_sb{pos % 4}")
            nc.vector.tensor_copy(rsb_sb[:], rsb_ps[:])
            o_ps = ptile([HD, P], sb, f"o_ps{sb}")
            nc.tensor.matmul(
                o_ps[:], v_sb[m][:, g * HD:(g + 1) * HD], est[:])
            if pos % 2 == 0:
                nc.vector.tensor_mul(
                    at_sb[0:HD, hh, m * P:(m + 1) * P], o_ps[:],
                    rsb_sb[:])
            else:
                nc.vector.tensor_mul(odd_stage[:, hh, :], o_ps[:],
                                     rsb_sb[:])

        def chunk_attention(m, qn_t, rstd, stb, csb, trb=None):
            qrT, odd_stage = chunk_pre(m, qn_t, rstd, trb or stb)
            for pos in range(NH):
                attn_pos(m, qrT, odd_stage, pos, stb, csb)
            nc.gpsimd.dma_start(
                at_sb[HD:P, :, m * P:(m + 1) * P], odd_stage[:])

        yt_r = yt.rearrange("(mo p) t -> p mo t", p=P)
        wo_r = wo.rearrange("mo p ko j -> p mo ko j")

        def _P5_EMIT(half, banks):
            c0 = half * 256
            for mb in range(8):
                wo_m = wstream.tile([P, 4, 8, P], F16, tag="wo",
                                    name=f"wo_m{half}_{mb}")
                nc.sync.dma_start(wo_m[:], wo_r[:, mb * 4:(mb + 1) * 4, :, :])
                ys = ystage.tile([P, 4, 256], F16, tag="ys", name="ys")
                for sub in range(4):
                    mo = mb * 4 + sub
                    ps = ptile([P, 256], banks[mo % len(banks)], f"ps_y{mo}")
                    for k in range(8):
                        nc.tensor.matmul(ps[:], wo_m[:, sub, k, :],
                                         at_sb[:, k, c0:c0 + 256],
                                         start=(k == 0), stop=(k == 7))
                    nc.scalar.copy(ys[:, sub, :], ps[:])
                nc.gpsimd.dma_start(yt_r[:, mb * 4:(mb + 1) * 4, c0:c0 + 256],
                                    ys[:])

        phase_a_half([0, 1])
        qs0 = q_stats(0)
        qs1 = q_stats(1)
        # rope + transposes for chunk 0 now (psum banks 0,1 freed by A1)
        qrT0, odd0 = chunk_pre(0, *qs0, trb=[0, 1])
        # phase A half 2 with chunk-0 attention interleaved: 2 heads per
        # weight batch keep the softmax chains flowing while PE stays dense.
        for kb in range(8):
            wq_k = wstream.tile([P, 4, QD], F16, tag="wq", name=f"wq_kB{kb}")
            nc.sync.dma_start(wq_k[:], wq_r[:, kb * 4:(kb + 1) * 4, :])
            for kk in range(4):
                k = kb * 4 + kk
                for m in (2, 3):
                    for s in range(2):
                        nc.tensor.matmul(
                            ps_a[m][s][:], xt_sb[:, k, m * P:(m + 1) * P],
                            wq_k[:, kk, s * 512:(s + 1) * 512],
                            start=(k == 0), stop=(k == KO - 1))
            attn_pos(0, qrT0, odd0, 2 * kb, [0, 1], [2, 3])
            attn_pos(0, qrT0, odd0, 2 * kb + 1, [0, 1], [2, 3])
        nc.gpsimd.dma_start(at_sb[HD:P, :, 0:P], odd0[:])
        qs2 = q_stats(2)
        qs3 = q_stats(3)
        chunk_attention(1, *qs1, stb=[0, 1, 4, 5], csb=[2, 3, 6, 7])
        chunk_attention(2, *qs2, stb=[0, 1, 4, 5], csb=[2, 3, 6, 7])
        _P5_EMIT(0, [0, 1, 2, 3])
        chunk_attention(3, *qs3, stb=[4, 5, 6, 7], csb=[6, 7, 4, 5])
        _P5_EMIT(1, [2, 3, 0, 1])

    nc.finalize()
    return nc


def host_inputs(inputs, core):
    """Build the per-core DRAM input map from full problem inputs."""
    hs = np.asarray(inputs["hidden_states"], np.float32)
    am = np.asarray(inputs["attention_mask"], np.float32)
    cos = np.asarray(inputs["cos"], np.float32)
    sin = np.asarray(inputs["sin"], np.float32)
    Wqkv = np.asarray(inputs["Wqkv"], np.float32)
    Wo = np.asarray(inputs["Wo"], np.float32)
    qw = np.asarray(inputs["q_norm_w"], np.float32)
    kw = np.asarray(inputs["k_norm_w"], np.float32)

    LS = 256
    ls = slice(core * LS, (core + 1) * LS)
    X = hs[:, ls, :].reshape(T, HID)
    xt = np.ascontiguousarray(X.T).astype(np.float16)
    cos_c = cos[:, ls, :].reshape(T, HD)
    sin_c = sin[:, ls, :].reshape(T, HD)
    sq = float(HD) ** -0.25  # sqrt(1/sqrt(HD)) = sqrt(1/8)
    swap = np.concatenate([np.arange(32, 64), np.arange(0, 32)])
    sign = np.concatenate([-np.ones(32, np.float32), np.ones(32, np.float32)])
    m = {
        "xt": xt,
        "cwq": np.ascontiguousarray(cos_c * qw[None, :] * sq),
        "swq": np.ascontiguousarray(sin_c * qw[swap][None, :] * sign[None, :] * sq),
        "cwk": np.ascontiguousarray(cos_c * kw[None, :] * sq),
        "swk": np.ascontiguousarray(sin_c * kw[swap][None, :] * sign[None, :] * sq),
        "wq": np.ascontiguousarray(
            Wqkv[:, :QD].reshape(HID, NH, HD)[:, PERM, :]
            .reshape(HID, QD)).astype(np.float16),
        "wkv": np.ascontiguousarray(Wqkv[:, QD:]).astype(np.float16),
        "wo": np.ascontiguousarray(
            Wo.reshape(NH, HD, HID)[PERM].reshape(QD, HID)
              .reshape(8, P, 32, P).transpose(2, 1, 0, 3)).astype(np.float16),
        "mask": np.clip(np.ascontiguousarray(am[0, 0, :P, :P].T),
                        -60000.0, None).astype(np.float16),
    }
    return m


def assemble_output(yts):
    """yts: list of 8 [4096, 512] fp16 arrays -> [2, 2048, 4096] f32."""
    out = np.empty((2, 2048, HID), np.float32)
    for c, yt_ in enumerate(yts):
        sl = yt_.astype(np.float32).T.reshape(2, 256, HID)
        out[:, c * 256:(c + 1) * 256, :] = sl
    return out




_NC_CACHE = {}


def _get_nc():
    if "nc" not in _NC_CACHE:
        _NC_CACHE["nc"] = build_nc()
    return _NC_CACHE["nc"]


def _run(inputs, trace=False):
    from concourse.bass_utils import run_bass_kernel_spmd
    nc = _get_nc()
    in_maps = [host_inputs(inputs, c) for c in range(8)]
    res = run_bass_kernel_spmd(nc, in_maps, core_ids=list(range(8)),
                               trace=trace)
    out = assemble_output([res.results[c]["yt"] for c in range(8)])
    return out, res


def kernel(**inputs):
    out, _ = _run(inputs, trace=False)
    return out


def _timed_runs(inputs, n=20):
    """Amortized per-execution wall time (ns) of the compiled SPMD body with
    device-resident inputs. Used by test.py; not part of the grading path."""
    import time
    import jax
    from jax.sharding import Mesh, PartitionSpec, NamedSharding
    from jax.experimental.shard_map import shard_map
    import concourse.bass2jax as b2j
    import concourse.mybir as _mb

    nc = _get_nc()
    in_maps = [host_inputs(inputs, c) for c in range(8)]
    n_cores = 8
    b2j.install_neuronx_cc_hook()
    pname = nc.partition_id_tensor.name if nc.partition_id_tensor else None
    in_names, out_names, out_avals, zero_outs = [], [], [], []
    for alloc in nc.m.functions[0].allocations:
        if not isinstance(alloc, _mb.MemoryLocationSet):
            continue
        name = alloc.memorylocations[0].name
        if alloc.kind == "ExternalInput":
            if name != pname:
                in_names.append(name)
        elif alloc.kind == "ExternalOutput":
            out_names.append(name)
            shape = tuple(alloc.tensor_shape)
            dtype = _mb.dt.np(alloc.dtype)
            out_avals.append(jax.core.ShapedArray(shape, dtype))
            zero_outs.append(np.zeros(shape, dtype))
    n_params = len(in_names)
    all_in = list(in_names) + list(out_names)
    if pname is not None:
        all_in.append(pname)

    def _body(*args):
        operands = list(args)
        if pname is not None:
            operands.append(b2j.partition_id_tensor())
        return tuple(b2j._bass_exec_p.bind(
            *operands, out_avals=tuple(out_avals), in_names=tuple(all_in),
            out_names=tuple(out_names), lowering_input_output_aliases=(),
            sim_require_finite=True, sim_require_nnan=True, nc=nc))

    devices = jax.devices()[:n_cores]
    mesh = Mesh(np.asarray(devices), ("core",))
    specs = (PartitionSpec("core"),) * (n_params + len(out_names))
    fn = jax.jit(shard_map(_body, mesh=mesh, in_specs=specs,
                           out_specs=(PartitionSpec("core"),) * len(out_names),
                           check_rep=False), keep_unused=True)
    per_core = [[np.asarray(m[nm]) for nm in in_names] for m in in_maps]
    concat_in = [np.concatenate([per_core[c][i] for c in range(n_cores)])
                 for i in range(n_params)]
    concat_zero = [np.zeros((n_cores * z.shape[0], *z.shape[1:]), z.dtype)
                   for z in zero_outs]
    sh = NamedSharding(mesh, PartitionSpec("core"))
    dev_in = [jax.device_put(a, sh) for a in concat_in + concat_zero]
    out = fn(*dev_in)
    jax.block_until_ready(out)
    best = None
    for _ in range(3):
        t0 = time.time()
        for _ in range(n):
            out = fn(*dev_in)
        jax.block_until_ready(out)
        dt = (time.time() - t0) / n * 1e9
        best = dt if best is None else min(best, dt)
    return best



# revision 10
# speedup vs baseline: 1.3797x; 1.3797x over previous
"""Trainium2 Bass kernel for nn_DFlashSelfAttention (block-sparse GQA attention).

Self-contained: builds the Bass module once, shards inputs over 8 NeuronCores
(sequence-parallel), runs via run_bass_kernel_spmd, reassembles full output.

Sharding: sequence-parallel over L (2048 -> 8 cores x 256 positions).
Attention is block-diagonal with BLOCK=16, so positions never interact
across 16-blocks; a 256-position slice (16 blocks) is fully independent.

Per-core pipeline (T = 512 rows = 2 batches x 256 positions):
  phase B: KV = X @ Wkv (4 psum banks), k-rope staggered per chunk.
  phase A: Q = X @ Wq chunk-sequential (2 banks/chunk, alternating pairs);
    wq is SBUF-resident (streamed once during phase B).  Attention for
    chunk m-1 is woven into chunk m's matmul stream.
  attention (per 128-chunk, per head): S = krT.T @ qrT (one N=128 matmul),
    exp on ACT (unmasked; RMS-normed q,k bound logits so fp16 is safe),
    0/1 block-causal mask applied multiplicatively on Pool, then one
    N=65 matmul [o | colsum] = est.T @ [v | 1], reciprocal on DVE,
    per-partition normalize on ACT -> A [q, (h d)], PE-transpose to A^T.
  emit: Y^T = Wo^T @ A^T in 4 quadrants (mo-half x t-half); wo is
    SBUF-resident, aliased onto the dead wkv / wq regions (loaded once).

All matmul operands fp16 (1 cy/row); accumulation and softmax math fp32.
RMS-norm weights and the attention scale fold into host rope tables.
"""

import sys as _sys
for _p in ("/opt/trn_rl_repo",):
    if _p not in _sys.path:
        _sys.path.insert(0, _p)

import ml_dtypes
import numpy as np

import concourse.bass as bass
import concourse.mybir as mybir
import concourse.tile as tile
from concourse import bacc
from concourse.masks import make_identity

F32 = mybir.dt.float32
F16 = mybir.dt.float16

P = 128
HID = 4096
KO = HID // P          # 32 k-chunks over hidden
T = 512                # rows per core: 2 batches x 256 positions
NM = T // P            # 4 t-chunks
NH = 16
NKV = 4
HD = 64
QD = NH * HD           # 1024
KVD = 2 * NKV * HD     # 512 (k 256 | v 256)
EPS = 1e-6

# Q-head permutation: position p holds original head PERM[p]. Even positions
# carry heads whose KV head is even (partition half 0), odd positions heads
# with odd KV head (half 64) — so S-matmul operands share a base partition.
PERM = [0, 4, 1, 5, 2, 6, 3, 7, 8, 12, 9, 13, 10, 14, 11, 15]


def build_nc(name="dfa"):
    nc = bacc.Bacc(None, target_bir_lowering=False, name=name)

    xt = nc.dram_tensor("xt", [HID, T], F16, kind="ExternalInput")
    wq = nc.dram_tensor("wq", [HID, QD], F16, kind="ExternalInput")
    wkv = nc.dram_tensor("wkv", [HID, KVD], F16, kind="ExternalInput")
    wo = nc.dram_tensor("wo", [32, P, 8, P], F16, kind="ExternalInput")
    cwq = nc.dram_tensor("cwq", [T, HD], F32, kind="ExternalInput")
    swq = nc.dram_tensor("swq", [T, HD], F32, kind="ExternalInput")
    cwk = nc.dram_tensor("cwk", [T, HD], F32, kind="ExternalInput")
    swk = nc.dram_tensor("swk", [T, HD], F32, kind="ExternalInput")
    mask = nc.dram_tensor("mask", [P, P], F16, kind="ExternalInput")
    yt = nc.dram_tensor("yt", [HID, T], F16, kind="ExternalOutput")

    from contextlib import ExitStack
    with tile.TileContext(nc) as tc, ExitStack() as ctx:
        consts = ctx.enter_context(tc.tile_pool(name="consts", bufs=1))
        xt_pool = ctx.enter_context(tc.tile_pool(name="xt", bufs=1))
        wbig = ctx.enter_context(tc.tile_pool(name="wbig", bufs=1))
        wkv_pool = ctx.enter_context(tc.tile_pool(name="wkvp", bufs=1))
        acts = ctx.enter_context(tc.tile_pool(name="acts", bufs=1))
        rope_tmp = ctx.enter_context(tc.tile_pool(name="rope_tmp", bufs=1))
        attn_tmp = ctx.enter_context(tc.tile_pool(name="attn_tmp", bufs=1))
        ystage = ctx.enter_context(tc.tile_pool(name="ystage", bufs=2))
        pp = ctx.enter_context(tc.tile_pool(name="pp", bufs=1, space="PSUM"))

        def ptile(shape, bank, name, dtype=F32):
            # pad the last free dim so each tag slot is exactly one 2KB bank
            tgt = 512 if dtype == F32 else 1024
            mid = 1
            for d in shape[1:-1]:
                mid *= d
            padded = list(shape[:-1]) + [tgt // mid]
            return pp.tile(shape, dtype, tag=f"b{bank}", name=name,
                           padded_shape=padded)

        # ---- resident SBUF tensors & leading DMAs ----
        xt_sb = xt_pool.tile([P, KO, T], F16)
        wq_sb = wbig.tile([P, KO, QD], F16, tag="wbig", name="wq_sb")
        wkv_sb = wkv_pool.tile([P, KO, KVD], F16, tag="wkv", name="wkv_sb")
        xt_r = xt.rearrange("(ko p) t -> p ko t", p=P)
        wkv_r = wkv.rearrange("(kb p) n -> p kb n", p=P)
        wq_r = wq.rearrange("(kb p) n -> p kb n", p=P)
        # first k-chunk of x and wkv land first so phase B starts ASAP
        nc.sync.dma_start(xt_sb[:, 0:1, :], xt_r[:, 0:1, :])
        nc.sync.dma_start(wkv_sb[:, 0:1, :], wkv_r[:, 0:1, :])
        nc.sync.dma_start(xt_sb[:, 1:4, :], xt_r[:, 1:4, :])
        nc.sync.dma_start(wkv_sb[:, 1:4, :], wkv_r[:, 1:4, :])
        for q in range(1, 8):
            nc.sync.dma_start(xt_sb[:, q * 4:(q + 1) * 4, :],
                              xt_r[:, q * 4:(q + 1) * 4, :])
            nc.sync.dma_start(wkv_sb[:, q * 4:(q + 1) * 4, :],
                              wkv_r[:, q * 4:(q + 1) * 4, :])
        for kb in range(8):
            nc.sync.dma_start(wq_sb[:, kb * 4:(kb + 1) * 4, :],
                              wq_r[:, kb * 4:(kb + 1) * 4, :])

        # ---- constants ----
        ident = consts.tile([P, P], F16)
        make_identity(nc, ident)
        eps_t = consts.tile([P, 1], F32)
        nc.vector.memset(eps_t, EPS)
        mask_sb = consts.tile([P, P], F16)   # 0/1 allowed mask [key, query]
        nc.sync.dma_start(mask_sb[:], mask[:])
        tabs = {}
        for nm_, dr_ in (("cwq", cwq), ("swq", swq), ("cwk", cwk),
                         ("swk", swk)):
            tt = consts.tile([P, NM, HD], F32, tag=nm_, name=nm_)
            nc.sync.dma_start(tt[:], dr_.rearrange("(m p) d -> p m d", p=P))
            tabs[nm_] = tt

        # wo resident: first 16 mo-batches alias the wkv region, last 16 the
        # wq region (both dead by the time the DMA fires; Tile handles WAR).
        wo_r = wo.rearrange("mo p ko j -> p mo ko j")
        wo_a = wkv_pool.tile([P, 16, 8, P], F16, tag="wkv", name="wo_a")
        wo_b = wbig.tile([P, 16, 8, P], F16, tag="wbig", name="wo_b")

        # ---- phase B: KV = X @ Wkv (banks 4-7) ----
        ps_b = [ptile([P, KVD], 4 + m, f"psb{m}") for m in range(NM)]
        for kb in range(8):
            for kk in range(4):
                k = kb * 4 + kk
                for m in range(NM):
                    nc.tensor.matmul(
                        ps_b[m][:], xt_sb[:, k, m * P:(m + 1) * P],
                        wkv_sb[:, k, :],
                        start=(k == 0), stop=(k == KO - 1))

        # wo_a DMA issues now on the sync queue: it waits on the wkv-region
        # WAR sem, then streams during phase A.
        for i in range(4):
            nc.sync.dma_start(wo_a[:, i * 4:(i + 1) * 4, :, :],
                              wo_r[:, i * 4:(i + 1) * 4, :, :])

        # ---- phase B copy-outs ----
        k_nat = []
        v_aug = []
        for m in range(NM):
            kn = acts.tile([P, NKV, HD], F32, tag=f"knat{m}", name=f"knat{m}")
            nc.vector.tensor_copy(
                kn[:], ps_b[m][:, 0:256].rearrange("p (h d) -> p h d", d=HD))
            k_nat.append(kn)
            vv = acts.tile([P, NKV, HD + 1], F16, tag=f"v{m}", name=f"v{m}")
            nc.vector.tensor_copy(
                vv[:, :, 0:HD],
                ps_b[m][:, 256:512].rearrange("p (g d) -> p g d", d=HD))
            nc.vector.memset(vv[:, :, HD:HD + 1], 1.0)
            v_aug.append(vv)

        def rope_stats(src_t, nh, tag):
            """x^2 sum -> rstd [P, nh] f32."""
            sq = rope_tmp.tile([P, nh, HD], F32, tag=f"sq{nh}", name="sq",
                               bufs=2)
            nc.vector.tensor_mul(sq[:], src_t[:], src_t[:])
            rstd = rope_tmp.tile([P, nh], F32, tag=tag, bufs=4, name=tag)
            nc.vector.reduce_sum(rstd[:], sq[:], axis=mybir.AxisListType.X)
            nc.scalar.activation(rstd[:], rstd[:],
                                 mybir.ActivationFunctionType.Sqrt,
                                 bias=eps_t[:], scale=1.0 / HD)
            nc.vector.reciprocal(rstd[:], rstd[:])
            return rstd

        def rope_apply(src_t, rstd, m, nh, ctab, stab, out_tag):
            """(src*rstd) rotary -> fp16 tile [P, nh*HD]. DVE/Pool only."""
            qn = rope_tmp.tile([P, nh, HD], F32, tag=f"qn{nh}", name="qn")
            nc.vector.tensor_mul(qn[:], src_t[:],
                                 rstd[:, :, None].to_broadcast((P, nh, HD)))
            o1 = rope_tmp.tile([P, nh, HD], F32, tag=f"o1{nh}", name="o1")
            nc.vector.tensor_mul(o1[:], qn[:],
                                 ctab[:, m, None, :].to_broadcast((P, nh, HD)))
            o2 = rope_tmp.tile([P, nh, HD], F32, tag=f"o2{nh}", name="o2")
            H2 = HD // 2
            nc.gpsimd.tensor_mul(
                o2[:, :, 0:H2], qn[:, :, H2:HD],
                stab[:, m, None, 0:H2].to_broadcast((P, nh, H2)))
            nc.gpsimd.tensor_mul(
                o2[:, :, H2:HD], qn[:, :, 0:H2],
                stab[:, m, None, H2:HD].to_broadcast((P, nh, H2)))
            outt = rope_tmp.tile([P, nh * HD], F16, tag=out_tag, bufs=2,
                                 name=out_tag)
            nc.vector.tensor_add(
                outt[:], o1[:].rearrange("p h d -> p (h d)"),
                o2[:].rearrange("p h d -> p (h d)"))
            return outt

        # krT_sb: slot m*2+g2 = transpose of kr_m cols [g2*128,(g2+1)*128];
        # partitions 0:64 = kv head 2*g2, 64:128 = kv head 2*g2+1.
        krT_sb = acts.tile([P, 8, P], F16, tag="krT", name="krT_sb")
        kr_tiles = {}

        def krope(m):
            k_rstd = rope_stats(k_nat[m], NKV, f"krstd{m}")
            kr_tiles[m] = rope_apply(k_nat[m], k_rstd, m, NKV, tabs["cwk"],
                                     tabs["swk"], "krout")

        def krT_gen(m):
            pt = ptile([P, 2, P], 7, f"ktp{m}", F16)
            for g2 in range(2):
                nc.tensor.matmul(pt[:, g2, :],
                                 kr_tiles[m][:, g2 * P:(g2 + 1) * P],
                                 ident[:], is_transpose=True)
            nc.vector.tensor_copy(krT_sb[:, 2 * m:2 * m + 2, :], pt[:])

        # A^T accumulator [P, 8, T]: slot j holds permuted positions
        # (2j, 2j+1); partition = (pos%2)*64 + d; col t.
        at_sb = acts.tile([P, 8, T], F16, tag="at", name="at_sb")

        # ---- per-chunk attention state ----
        qn_tiles = {}
        qr_tiles = {}
        qrT_tiles = {}
        a_tiles = {}

        def q_stats(m, ps_a):
            qn_t = acts.tile([P, NH, HD], F32, tag=f"qnat{m % 2}",
                             name=f"qnat{m}", bufs=1)
            for s in range(2):
                nc.vector.tensor_copy(
                    qn_t[:, s * 8:(s + 1) * 8, :],
                    ps_a[s][:].rearrange("p (h d) -> p h d", d=HD))
            qn_tiles[m] = qn_t

        def qrope(m):
            rstd = rope_stats(qn_tiles[m], NH, f"qrstd{m}")
            qr_tiles[m] = rope_apply(qn_tiles[m], rstd, m, NH, tabs["cwq"],
                                     tabs["swq"], "qrout")

        def qrT_half(m, h):
            """Transpose qr(m) cols [h*512,(h+1)*512] -> qrT slots 4h..4h+3."""
            if h == 0:
                qrT_tiles[m] = attn_tmp.tile([P, 8, P], F16, tag="qrT",
                                             bufs=2, name=f"qrT{m}")
                a_tiles[m] = attn_tmp.tile([P, NH, HD], F16, tag="A", bufs=2,
                                           name=f"A{m}")
            pt = ptile([P, 4, P], 7, f"qtp{m}_{h}", F16)
            for j in range(4):
                hh = 4 * h + j
                nc.tensor.matmul(pt[:, j, :],
                                 qr_tiles[m][:, hh * P:(hh + 1) * P],
                                 ident[:], is_transpose=True)
            nc.vector.tensor_copy(qrT_tiles[m][:, 4 * h:4 * h + 4, :], pt[:])

        est_tiles = {}
        po_tiles = {}

        def attn_S(m, pos, stb):
            h = PERM[pos]
            base = (pos % 2) * HD
            hh = pos // 2
            lq = qrT_tiles[m][base:base + HD, hh, :]
            lk = krT_sb[base:base + HD, 2 * m + (h // 4) // 2, :]
            st_ps = ptile([P, P], stb, f"st{stb}")
            nc.tensor.matmul(st_ps[:], lk, lq)
            eraw = attn_tmp.tile([P, P], F16, tag=f"eraw{pos % 3}", bufs=1,
                                 name=f"eraw{pos % 3}")
            nc.scalar.activation(eraw[:], st_ps[:],
                                 mybir.ActivationFunctionType.Exp)
            est = attn_tmp.tile([P, P], F16, tag=f"est{pos % 3}", bufs=1,
                                name=f"est{pos % 3}")
            nc.gpsimd.tensor_mul(est[:], eraw[:], mask_sb[:])
            est_tiles[(m, pos)] = est

        def attn_O(m, pos, ob):
            h = PERM[pos]
            g = h // 4
            est = est_tiles.pop((m, pos))
            po = ptile([P, HD + 1], ob, f"po{ob}")
            nc.tensor.matmul(po[:], est[:], v_aug[m][:, g, :])
            rs = attn_tmp.tile([P, 1], F32, tag=f"rs{pos % 4}", bufs=1,
                               name=f"rs{pos % 4}")
            nc.vector.reciprocal(rs[:], po[:, HD:HD + 1])
            nc.scalar.mul(a_tiles[m][:, pos, :], po[:, 0:HD], rs[:, 0:1])

        def at_gen(m, bank):
            """8 PE transposes of A(m) -> at_sb cols [m*128,(m+1)*128]."""
            pt = ptile([P, 8, P], bank, f"atp{m}", F16)
            af = a_tiles[m][:].rearrange("p h d -> p (h d)")
            for j in range(8):
                nc.tensor.matmul(pt[:, j, :], af[:, j * P:(j + 1) * P],
                                 ident[:], is_transpose=True)
            nc.vector.tensor_copy(at_sb[:, :, m * P:(m + 1) * P], pt[:])

        # ---- phase A driver with weave slots ----
        def phase_a_chunk(m, weave):
            """32 kk-steps; each: 2 matmuls (s=0,1). weave[step] callables
            run before that step's matmuls."""
            bp = 2 * (m % 2)
            ps = [ptile([P, 512], bp + s, f"psa{m}_{s}") for s in range(2)]
            for step in range(32):
                for fn in weave.get(step, ()):
                    fn()
                kb, kk = divmod(step, 4)
                k = step
                for s in range(2):
                    nc.tensor.matmul(
                        ps[s][:], xt_sb[:, k, m * P:(m + 1) * P],
                        wq_sb[:, k, s * 512:(s + 1) * 512],
                        start=(k == 0), stop=(k == KO - 1))
            return ps

        def wadd(w, step, fn):
            w.setdefault(step, []).append(fn)

        def attn_sched(w, spill, c, sS0, sO0, X, per, nsteps, spill_X=None):
            """S(c,i) at sS0+i//per, O(c,i) at sO0+i//per; late entries go
            to `spill` (next window, step - nsteps). O uses banks (6, X)
            in-window; spilled O's always use (6, X2=X) which the caller
            guarantees free next window."""
            def add(step, fn, spilled_fn=None):
                if step < nsteps:
                    w.setdefault(step, []).append(fn)
                else:
                    spill.setdefault(step - nsteps, []).append(
                        spilled_fn or fn)
            for i in range(NH):
                add(sS0 + i // per,
                    (lambda c=c, i=i, b=(4, 5)[i % 2]: attn_S(c, i, b)))
                # spilled O's always use bank 6: the in-window X bank may
                # belong to the next window's GEMM or ps_y rotation.
                sx = (6, spill_X)[i % 2] if spill_X is not None else 6
                add(sO0 + i // per,
                    (lambda c=c, i=i, b=(6, X)[i % 2]: attn_O(c, i, b)),
                    (lambda c=c, i=i, b=sx: attn_O(c, i, b)))

        # ---- schedule ----
        # Window w = chunk-w GEMM hosting attn(w-1).  Template (427ns steps):
        #   0: tail O's of attn(w-2) (bank 6); 1: q_stats/qrope(w-1);
        #   4: at_gen(w-3) on a freed GEMM bank; 14: krT_gen(w-1);
        #   15/17: qrT halves; 16+i: S(i); 18+i: O(i); 18: krope(w).
        krope(0)
        ps_a = {}
        w0 = {18: [lambda: krope(1)]}
        ps_a[0] = phase_a_chunk(0, w0)

        def mk_qstats(c):
            def fn():
                q_stats(c, ps_a[c])
                qrope(c)
            return fn

        w1 = {1: [mk_qstats(0)],
              14: [lambda: krT_gen(0)],
              15: [lambda: qrT_half(0, 0)],
              17: [lambda: qrT_half(0, 1)],
              18: [lambda: krope(2)]}
        spill = {}
        attn_sched(w1, spill, 0, 16, 18, 1, 1, 32)
        ps_a[1] = phase_a_chunk(1, w1)

        w2 = {k_: list(v_) for k_, v_ in spill.items()}
        wadd(w2, 1, mk_qstats(1))
        wadd(w2, 4, lambda: at_gen(0, 2))
        wadd(w2, 14, lambda: krT_gen(1))
        wadd(w2, 15, lambda: qrT_half(1, 0))
        wadd(w2, 17, lambda: qrT_half(1, 1))
        wadd(w2, 18, lambda: krope(3))
        spill = {}
        attn_sched(w2, spill, 1, 16, 18, 3, 1, 32)
        ps_a[2] = phase_a_chunk(2, w2)

        w3 = {k_: list(v_) for k_, v_ in spill.items()}
        wadd(w3, 1, mk_qstats(2))
        wadd(w3, 4, lambda: at_gen(1, 0))
        wadd(w3, 14, lambda: krT_gen(2))
        wadd(w3, 15, lambda: qrT_half(2, 0))
        wadd(w3, 17, lambda: qrT_half(2, 1))
        spill = {}
        attn_sched(w3, spill, 2, 16, 18, 1, 1, 32)
        ps_a[3] = phase_a_chunk(3, w3)

        # wo_b aliases the wq region (dead after chunk 3's matmuls).
        for i in range(4):
            nc.sync.dma_start(wo_b[:, i * 4:(i + 1) * 4, :, :],
                              wo_r[:, 16 + i * 4:16 + (i + 1) * 4, :, :])

        # ---- emit quadrants ----
        yt_r = yt.rearrange("(mo p) t -> p mo t", p=P)

        def emit_quad(wo_sb, mo0, c0, ybanks, weave, ys_dve):
            for s in range(16):
                for fn in weave.get(s, ()):
                    fn()
                ps = ptile([P, 256], ybanks[s % len(ybanks)], f"psy{s}")
                for k in range(8):
                    nc.tensor.matmul(ps[:], wo_sb[:, s, k, :],
                                     at_sb[:, k, c0:c0 + 256],
                                     start=(k == 0), stop=(k == 7))
                if s % 4 == 0:
                    ys = ystage.tile([P, 4, 256], F16, tag="ys", name="ys")
                if ys_dve:
                    nc.vector.tensor_copy(ys[:, s % 4, :], ps[:])
                else:
                    nc.scalar.copy(ys[:, s % 4, :], ps[:])
                if s % 4 == 3:
                    nc.gpsimd.dma_start(
                        yt_r[:, mo0 + s - 3:mo0 + s + 1, c0:c0 + 256], ys[:])

        # Q1: mo 0..15, t 0:256 (853ns steps); hosts attn(2) tail + attn(3).
        wq1 = {k_: list(v_) for k_, v_ in spill.items()}
        wadd(wq1, 1, mk_qstats(3))
        wadd(wq1, 3, lambda: krT_gen(3))
        wadd(wq1, 6, lambda: qrT_half(3, 0))
        wadd(wq1, 7, lambda: qrT_half(3, 1))
        spill = {}
        attn_sched(wq1, spill, 3, 10, 12, 3, 2, 16, spill_X=3)
        emit_quad(wo_a, 0, 0, (0, 1, 2), wq1, ys_dve=False)

        # Q2: attn(3) tail + both remaining A^T transposes.
        wq2 = {k_: list(v_) for k_, v_ in spill.items()}
        wadd(wq2, 4, lambda: at_gen(2, 3))
        wadd(wq2, 6, lambda: at_gen(3, 7))
        emit_quad(wo_b, 16, 0, (0, 1, 2), wq2, ys_dve=False)
        emit_quad(wo_a, 0, 256, (0, 1, 2, 3), {}, ys_dve=False)
        emit_quad(wo_b, 16, 256, (0, 1, 2, 3), {}, ys_dve=False)

    nc.finalize()
    return nc


def host_inputs(inputs, core):
    """Build the per-core DRAM input map from full problem inputs."""
    hs = np.asarray(inputs["hidden_states"], np.float32)
    am = np.asarray(inputs["attention_mask"], np.float32)
    cos = np.asarray(inputs["cos"], np.float32)
    sin = np.asarray(inputs["sin"], np.float32)
    Wqkv = np.asarray(inputs["Wqkv"], np.float32)
    Wo = np.asarray(inputs["Wo"], np.float32)
    qw = np.asarray(inputs["q_norm_w"], np.float32)
    kw = np.asarray(inputs["k_norm_w"], np.float32)

    LS = 256
    ls = slice(core * LS, (core + 1) * LS)
    X = hs[:, ls, :].reshape(T, HID)
    xt = np.ascontiguousarray(X.T).astype(np.float16)
    cos_c = cos[:, ls, :].reshape(T, HD)
    sin_c = sin[:, ls, :].reshape(T, HD)
    sq = float(HD) ** -0.25  # sqrt(1/sqrt(HD)) = sqrt(1/8)
    swap = np.concatenate([np.arange(32, 64), np.arange(0, 32)])
    sign = np.concatenate([-np.ones(32, np.float32), np.ones(32, np.float32)])
    m = {
        "xt": xt,
        "cwq": np.ascontiguousarray(cos_c * qw[None, :] * sq),
        "swq": np.ascontiguousarray(sin_c * qw[swap][None, :] * sign[None, :] * sq),
        "cwk": np.ascontiguousarray(cos_c * kw[None, :] * sq),
        "swk": np.ascontiguousarray(sin_c * kw[swap][None, :] * sign[None, :] * sq),
        "wq": np.ascontiguousarray(
            Wqkv[:, :QD].reshape(HID, NH, HD)[:, PERM, :]
            .reshape(HID, QD)).astype(np.float16),
        "wkv": np.ascontiguousarray(Wqkv[:, QD:]).astype(np.float16),
        "wo": np.ascontiguousarray(
            Wo.reshape(NH, HD, HID)[PERM].reshape(QD, HID)
              .reshape(8, P, 32, P).transpose(2, 1, 0, 3)).astype(np.float16),
        # 0/1 allowed mask, [key, query] orientation
        "mask": np.ascontiguousarray(
            (am[0, 0, :P, :P].T == 0.0)).astype(np.float16),
    }
    return m


def assemble_output(yts):
    """yts: list of 8 [4096, 512] fp16 arrays -> [2, 2048, 4096] f32."""
    out = np.empty((2, 2048, HID), np.float32)
    for c, yt_ in enumerate(yts):
        sl = yt_.astype(np.float32).T.reshape(2, 256, HID)
        out[:, c * 256:(c + 1) * 256, :] = sl
    return out


_NC_CACHE = {}


def _get_nc():
    if "nc" not in _NC_CACHE:
        _NC_CACHE["nc"] = build_nc()
    return _NC_CACHE["nc"]


def _run(inputs, trace=False):
    from concourse.bass_utils import run_bass_kernel_spmd
    nc = _get_nc()
    in_maps = [host_inputs(inputs, c) for c in range(8)]
    res = run_bass_kernel_spmd(nc, in_maps, core_ids=list(range(8)),
                               trace=trace)
    out = assemble_output([res.results[c]["yt"] for c in range(8)])
    return out, res


def kernel(**inputs):
    out, _ = _run(inputs, trace=False)
    return out


def _timed_runs(inputs, n=20):
    """Amortized per-execution wall time (ns) of the compiled SPMD body with
    device-resident inputs. Used by test.py; not part of the grading path."""
    import time
    import jax
    from jax.sharding import Mesh, PartitionSpec, NamedSharding
    from jax.experimental.shard_map import shard_map
    import concourse.bass2jax as b2j
    import concourse.mybir as _mb

    nc = _get_nc()
    in_maps = [host_inputs(inputs, c) for c in range(8)]
    n_cores = 8
    b2j.install_neuronx_cc_hook()
    pname = nc.partition_id_tensor.name if nc.partition_id_tensor else None
    in_names, out_names, out_avals, zero_outs = [], [], [], []
    for alloc in nc.m.functions[0].allocations:
        if not isinstance(alloc, _mb.MemoryLocationSet):
            continue
        name = alloc.memorylocations[0].name
        if alloc.kind == "ExternalInput":
            if name != pname:
                in_names.append(name)
        elif alloc.kind == "ExternalOutput":
            out_names.append(name)
            shape = tuple(alloc.tensor_shape)
            dtype = _mb.dt.np(alloc.dtype)
            out_avals.append(jax.core.ShapedArray(shape, dtype))
            zero_outs.append(np.zeros(shape, dtype))
    n_params = len(in_names)
    all_in = list(in_names) + list(out_names)
    if pname is not None:
        all_in.append(pname)

    def _body(*args):
        operands = list(args)
        if pname is not None:
            operands.append(b2j.partition_id_tensor())
        return tuple(b2j._bass_exec_p.bind(
            *operands, out_avals=tuple(out_avals), in_names=tuple(all_in),
            out_names=tuple(out_names), lowering_input_output_aliases=(),
            sim_require_finite=True, sim_require_nnan=True, nc=nc))

    devices = jax.devices()[:n_cores]
    mesh = Mesh(np.asarray(devices), ("core",))
    specs = (PartitionSpec("core"),) * (n_params + len(out_names))
    fn = jax.jit(shard_map(_body, mesh=mesh, in_specs=specs,
                           out_specs=(PartitionSpec("core"),) * len(out_names),
                           check_rep=False), keep_unused=True)
    per_core = [[np.asarray(m[nm]) for nm in in_names] for m in in_maps]
    concat_in = [np.concatenate([per_core[c][i] for c in range(n_cores)])
                 for i in range(n_params)]
    concat_zero = [np.zeros((n_cores * z.shape[0], *z.shape[1:]), z.dtype)
                   for z in zero_outs]
    sh = NamedSharding(mesh, PartitionSpec("core"))
    dev_in = [jax.device_put(a, sh) for a in concat_in + concat_zero]
    out = fn(*dev_in)
    jax.block_until_ready(out)
    best = None
    for _ in range(3):
        t0 = time.time()
        for _ in range(n):
            out = fn(*dev_in)
        jax.block_until_ready(out)
        dt = (time.time() - t0) / n * 1e9
        best = dt if best is None else min(best, dt)
    return best
